# revision 33
# baseline (speedup 1.0000x reference)
"""Trainium2 Bass kernel for nn_DifferentiableSolver (batched box-QP ADMM).

Self-contained: shards the 32768-sample batch across 8 NeuronCores (data
parallel), precomputes per-sample iteration operators on-device, runs the
100 unrolled ADMM iterations on-device, gathers the full output.

Math (per sample, algebraically identical to the reference recursion):
  M = A A^T + eps I ; T = Minv A ; P = A^T T
  R' = -P/sigma ;  e = A^T(Minv b + Minv(A c)/sigma) - c/sigma
  iterate: x = R'w + w/sigma + e ; s = x+u ; z = clip(s,lb,ub) ;
           u = s-z ; w = 2z-s          (w = z - u)

v2: fp32r matmuls with block-diagonal stationary (4x fewer PE row-streams,
4x faster per row), PSUM-direct DVE transposes, elementwise chain on the
GPSIMD engine, batched multi-dim-AP DMAs in precompute (one scatter DMA
replaces up to 128 small ones), Gauss-Jordan column-split across DVE/Pool.
"""
import sys
for p in ("/opt/trn_rl_repo",):
    if p not in sys.path:
        sys.path.append(p)

import numpy as np
import bass_rust
import concourse.bass as bass
import concourse.bacc as bacc
import concourse.mybir as mybir
from concourse.tile import TileContext

SIGMA = 1.2
RHO = 1.0
JITTER = 1e-5
GJW = 33          # GJ row width: 16 M + 16 I + 1 rhs
F32 = mybir.dt.float32
F32R = mybir.dt.float32r
GJ_SPLIT = 50     # tableau columns handled by DVE (rest on Pool)


def cap(t_ap, off, dims):
    """Build a raw AP on the same underlying (possibly symbolic) tensor."""
    return bass_rust.AP(tensor=t_ap.tensor, offset=t_ap.offset + off,
                        ap=[tuple(d) for d in dims])


def build_kernel(nc: bass.Bass, NB: int, n_iters: int, use_for_i: bool = True,
                 tensors=None, debug_out=None):
    G = NB // 128
    NCH = G // 2                      # chunks of 2 groups
    CPC = min(8, NCH)                 # chunks per cluster
    NCL = NCH // CPC                  # clusters
    CG = 2 * CPC                      # groups per cluster
    if tensors is None:
        A_d = nc.dram_tensor("A", [NB, 16, 32], F32, kind="ExternalInput")
        b_d = nc.dram_tensor("b", [NB, 16], F32, kind="ExternalInput")
        c_d = nc.dram_tensor("c", [NB, 32], F32, kind="ExternalInput")
        lb_d = nc.dram_tensor("lb", [NB, 32], F32, kind="ExternalInput")
        ub_d = nc.dram_tensor("ub", [NB, 32], F32, kind="ExternalInput")
        x_d = nc.dram_tensor("x", [NB, 32], F32, kind="ExternalOutput")
        T_d = nc.dram_tensor("Tstage", [NB, 16, 32], F32, kind="Internal")
        M_d = nc.dram_tensor("Mstage", [NB, 16, 16], F32, kind="Internal")
    else:
        A_d, b_d, c_d, lb_d, ub_d, x_d, T_d, M_d = tensors
    AL = mybir.AluOpType
    AX = mybir.AxisListType

    with TileContext(nc) as tc:
        with tc.tile_pool(name="pers", bufs=1) as pers:
            # ---- persistent state ----
            R_sb = pers.tile([128, G * 1024], F32, tag="R")
            Wstat = pers.tile([128, G * 128], F32, tag="Wstat")
            X_sb = pers.tile([128, G * 32], F32, tag="X")     # pre/x accum
            U_sb = pers.tile([128, G * 32], F32, tag="U")
            lbT = pers.tile([128, G * 32], F32, tag="lbT")
            ubT = pers.tile([128, G * 32], F32, tag="ubT")
            eT = pers.tile([128, G * 32], F32, tag="eT")

            Pr = R_sb[:, :].ap[0][0]
            Pws = Wstat[:, :].ap[0][0]
            Px = X_sb[:, :].ap[0][0]

            # =========== PRECOMPUTE ===========
            with (
                tc.tile_pool(name="scr", bufs=1) as scr,
                tc.tile_pool(name="grp", bufs=1) as grp,
                tc.tile_pool(name="psC", bufs=2, space="PSUM") as psC,
                tc.tile_pool(name="psT", bufs=2, space="PSUM") as psT,
            ):
                c_sm = scr.tile([128, G * 32], F32, tag="c_sm")
                e_sm = scr.tile([128, G * 32], F32, tag="e_sm")
                stage = e_sm

                def load_sm(dst, src_d, w):
                    sp = dst[:, :].ap[0][0]
                    d = cap(dst[:, :], 0, [(sp, 128), (w, G), (1, w)])
                    s = src_d[:, :].rearrange("(g p) w -> p g w", p=128)
                    nc.sync.dma_start(d, s)

                load_sm(c_sm, c_d, 32)
                # lb/ub: load sample-major into stage, transpose to T-layout
                load_sm(stage, lb_d, 32)
                nc.vector.transpose(lbT[:, :], stage[:, :])
                load_sm(stage, ub_d, 32)
                nc.vector.transpose(ubT[:, :], stage[:, :])

                gjL = [grp.tile([128, 16 * GJW], F32, tag=f"gj{i}",
                                name=f"gj{i}") for i in range(2)]
                gjp = gjL[0][:, :].ap[0][0]
                prod = grp.tile([128, 16 * GJW], F32, tag="prod")
                Pp = prod[:, :].ap[0][0]
                rowbuf = grp.tile([128, GJW], F32, tag="rowk")
                rcp = grp.tile([128, 1], F32, tag="rcp")
                Ac16 = grp.tile([128, 16], F32, tag="Ac16")
                b16 = grp.tile([128, 16], F32, tag="b16")
                ered = grp.tile([128, 32], F32, tag="ered")
                MinvBD = grp.tile([128, 2048], F32, tag="MinvBD")
                Pmb = MinvBD[:, :].ap[0][0]
                ApeT = grp.tile([128, 512], F32, tag="ApeT")
                Pat = ApeT[:, :].ap[0][0]
                sc_t = grp.tile([128, 512], F32, tag="sc_t")
                Pst2 = sc_t[:, :].ap[0][0]
                nc.vector.memset(MinvBD[:, :], 0.0)
                AsmL = [grp.tile([128, 512], F32, tag=f"asm{i}",
                                 name=f"asm{i}") for i in range(1)]
                sc2L = [grp.tile([128, 1024], F32, tag=f"sc2{i}",
                                 name=f"sc2{i}") for i in range(1)]
                Ps2 = sc2L[0][:, :].ap[0][0]
                ApeL = [grp.tile([16, 1024], F32, tag=f"ape{i}",
                                 name=f"ape{i}") for i in range(1)]
                TpeL = [grp.tile([16, 1024], F32, tag=f"tpe{i}",
                                 name=f"tpe{i}") for i in range(1)]
                for g in range(G):
                    gs = slice(g * 32, (g + 1) * 32)
                    gj = gjL[g % 2]
                    Asm = AsmL[0]
                    sc2 = sc2L[0]
                    # loads: A sample-major, b
                    nc.sync.dma_start(
                        Asm[:, :],
                        A_d[128 * g:128 * (g + 1), :, :]
                        .rearrange("n m i -> n (m i)"))
                    nc.sync.dma_start(b16[:, :], b_d[128 * g:128 * (g + 1), :])
                    Pa = Asm[:, :].ap[0][0]

                    # ---- phase A: M = A A^T in-partition on DVE ----
                    for dd in range(16):
                        nmm = 16 - dd
                        a1 = cap(Asm[:, :], 0, [(Pa, 128), (32, nmm), (1, 32)])
                        a2 = cap(Asm[:, :], 32 * dd,
                                 [(Pa, 128), (32, nmm), (1, 32)])
                        pv = cap(prod[:, :], 0, [(Pp, 128), (32, nmm),
                                                 (1, 32)])
                        nc.vector.tensor_tensor(pv, a1, a2, AL.mult)
                        up = cap(gj[:, :], dd, [(gjp, 128), (GJW + 1, nmm)])
                        nc.vector.tensor_reduce(up, pv, axis=AX.X, op=AL.add)
                        if dd:
                            lo = cap(gj[:, :], GJW * dd,
                                     [(gjp, 128), (GJW + 1, nmm)])
                            nc.scalar.copy(lo, up)
                    # M += eps I
                    diag = cap(gj[:, :], 0, [(gjp, 128), (GJW + 1, 16)])
                    nc.vector.tensor_scalar_add(diag, diag, JITTER)
                    # identity block (cols 16:32)
                    iblk = cap(gj[:, :], 16, [(gjp, 128), (GJW, 16), (1, 16)])
                    nc.vector.memset(iblk, 0.0)
                    idg = cap(gj[:, :], 16, [(gjp, 128), (GJW + 1, 16)])
                    nc.vector.memset(idg, 1.0)
                    # rhs col: gj[p, 17m+16] = b + (A c)/sigma
                    Pc = c_sm[:, :].ap[0][0]
                    cbc = cap(c_sm[:, :], 32 * g, [(Pc, 128), (0, 16), (1, 32)])
                    nc.vector.tensor_tensor(
                        cap(prod[:, :], 0, [(Pp, 128), (32, 16), (1, 32)]),
                        cap(Asm[:, :], 0, [(Pa, 128), (32, 16), (1, 32)]),
                        cbc, AL.mult)
                    nc.vector.tensor_reduce(
                        Ac16[:, :],
                        cap(prod[:, :], 0, [(Pp, 128), (32, 16), (1, 32)]),
                        axis=AX.X, op=AL.add)
                    rhscol = cap(gj[:, :], 32, [(gjp, 128), (GJW, 16)])
                    nc.vector.scalar_tensor_tensor(rhscol, Ac16[:, :],
                                                   1.0 / SIGMA, b16[:, :],
                                                   AL.mult, AL.add)

                    # ---- phase B: Gauss-Jordan on [M | rhs] ----
                    for k in range(16):
                        nc.vector.reciprocal(
                            rcp[:, :], gj[:, GJW * k + k: GJW * k + k + 1])
                        nc.vector.tensor_scalar(rowbuf[:, :],
                                                gj[:, GJW * k: GJW * (k + 1)],
                                                rcp[:, 0:1], None, AL.mult)
                        colk = cap(gj[:, :], k, [(gjp, 128), (GJW, 16)])
                        colk_b = colk.unsqueeze(2).broadcast_to([128, 16, GJW])
                        rowk_b = rowbuf[:, :].unsqueeze(1) \
                            .broadcast_to([128, 16, GJW])
                        prod_v = cap(prod[:, :], 0,
                                     [(Pp, 128), (GJW, 16), (1, GJW)])
                        gj3 = cap(gj[:, :], 0, [(gjp, 128), (GJW, 16),
                                                (1, GJW)])
                        nc.vector.tensor_tensor(prod_v, colk_b, rowk_b,
                                                AL.mult)
                        nc.vector.tensor_sub(gj3, gj3, prod_v)
                        nc.vector.tensor_copy(gj[:, GJW * k: GJW * (k + 1)],
                                              rowbuf[:, :])

                    # Minv -> DRAM, then block-diag stationary + T on PE
                    nc.sync.dma_start(
                        M_d[128 * g:128 * (g + 1), :, :]
                        .rearrange("s m n -> s (m n)"),
                        cap(gj[:, :], 16, [(gjp, 128), (GJW, 16), (1, 16)]))
                    Mr = M_d[128 * g:128 * (g + 1), :, :] \
                        .rearrange("(t h) m n -> h m t n", h=8)
                    for h in range(8):
                        dst = cap(MinvBD[:, :], 16 * h * Pmb + 16 * h,
                                  [(Pmb, 16), (128, 16), (1, 16)])
                        nc.sync.dma_start(dst, Mr[h])
                    nc.sync.dma_start(
                        cap(ApeT[:, :], 0, [(Pat, 128), (32, 16), (1, 32)]),
                        A_d[128 * g:128 * (g + 1), :, :]
                        .rearrange("(t h) m i -> (h m) t i", h=8))
                    ps_t = psT.tile([128, 512], F32, tag="pst")
                    for t in range(16):
                        nc.tensor.matmul(ps_t[:, 32 * t:32 * t + 32],
                                         MinvBD[:, 128 * t:128 * (t + 1)],
                                         ApeT[:, 32 * t:32 * t + 32],
                                         start=True, stop=True)
                    # sc_t = -T/sigma ; stage to DRAM for phase C
                    nc.vector.tensor_scalar(sc_t[:, 0:256], ps_t[:, 0:256],
                                            -1.0 / SIGMA, None, AL.mult)
                    nc.scalar.mul(sc_t[:, 256:512], ps_t[:, 256:512],
                                  -1.0 / SIGMA)
                    nc.sync.dma_start(
                        T_d[128 * g:128 * (g + 1), :, :]
                        .rearrange("(t h) m i -> (h m) t i", h=8),
                        cap(sc_t[:, :], 0, [(Pst2, 128), (32, 16), (1, 32)]))
                    # e = A^T e16 - c/sigma;  e16 = Minv rhs (gj col 16)
                    e16bc = cap(gj[:, :], 32, [(gjp, 128), (GJW, 16), (0, 32)])
                    nc.vector.tensor_tensor(
                        cap(prod[:, :], 0, [(Pp, 128), (32, 16), (1, 32)]),
                        cap(Asm[:, :], 0, [(Pa, 128), (32, 16), (1, 32)]),
                        e16bc, AL.mult)
                    nc.vector.tensor_reduce(
                        ered[:, :],
                        cap(prod[:, :], 0, [(Pp, 128), (1, 32), (32, 16)]),
                        axis=AX.X, op=AL.add)
                    nc.vector.scalar_tensor_tensor(
                        e_sm[:, gs], c_sm[:, gs], -1.0 / SIGMA, ered[:, :],
                        AL.mult, AL.add)

                    # ---- phase C: R = A^T * (-T/sigma) on PE ----
                    for cb in range(4):
                        Ape = ApeL[0]
                        Tpe = TpeL[0]
                        Pap = Ape[:, :].ap[0][0]
                        Ptp = Tpe[:, :].ap[0][0]
                        # Ape[m, 32s+i] = A[32cb+s, m, i]  (s = 4j+qt)
                        s0 = 128 * g + 32 * cb
                        srcA = A_d[s0:s0 + 32, :, :] \
                            .rearrange("s m i -> m s i")
                        dstA2 = cap(Ape[:, :], 0,
                                    [(Pap, 16), (32, 32), (1, 32)])
                        nc.sync.dma_start(dstA2, srcA)
                        # Tpe[m, 32s+i] = -T/sigma of same samples
                        srcT = T_d[s0:s0 + 32, :, :] \
                            .rearrange("s m i -> m s i")
                        dstT = cap(Tpe[:, :], 0,
                                   [(Ptp, 16), (32, 32), (1, 32)])
                        nc.sync.dma_start(dstT, srcT)
                        ps_c = psC.tile([128, 1024], F32, tag="psc")
                        for j in range(8):
                            po = slice(128 * j, 128 * (j + 1))
                            nc.tensor.matmul(ps_c[:, po], Ape[:, po],
                                             Tpe[:, po], start=True, stop=True)
                        # de-diag copies: sc2[32qt+a, 256cb+32j+k] =
                        #   ps_c[32qt+a, 128j+32qt+k]
                        Ppc = ps_c[:, :].ap[0][0]
                        for qt in range(4):
                            src = cap(ps_c[:, :], 32 * qt * Ppc + 32 * qt,
                                      [(Ppc, 32), (128, 8), (1, 32)])
                            dst = cap(sc2[:, :], 32 * qt * Ps2 + 256 * cb,
                                      [(Ps2, 32), (32, 8), (1, 32)])
                            if qt % 2 == 0:
                                nc.vector.tensor_copy(dst.bitcast(F32R), src)
                            else:
                                nc.scalar.copy(dst.bitcast(F32R), src)
                    # R-scatter: 16 3-dim DMAs per group (qt, cb) x (a, j, k)
                    for qt in range(4):
                        for cb in range(4):
                            src = cap(sc2[:, :], 32 * qt * Ps2 + 256 * cb,
                                      [(Ps2, 32), (32, 8), (1, 32)])
                            dst = cap(R_sb[:, :],
                                      32 * cb * Pr + 1024 * g + 32 * qt,
                                      [(Pr, 32), (128, 8), (1, 32)])
                            nc.sync.dma_start(dst, src)

                # eT from accumulated e_sm
                nc.vector.transpose(eT[:, :], e_sm[:, :])
                # W0 = clip(0, lb, ub) in T layout -> stage as W-init
                nc.vector.memset(stage[:, :], 0.0)
                nc.vector.tensor_max(stage[:, :], stage[:, :], lbT[:, :])
                nc.vector.tensor_tensor(stage[:, :], stage[:, :], ubT[:, :],
                                        AL.min)
                nc.vector.memset(U_sb[:, :], 0.0)
                # X = pre = W0/sigma + eT
                nc.vector.scalar_tensor_tensor(X_sb[:, :], stage[:, :],
                                               1.0 / SIGMA, eT[:, :], AL.mult,
                                               AL.add)
                # Wstat init: zero + 4 block-diag copies from W0 (=stage)
                zt = grp.tile([128, 1], F32, tag="zt")
                nc.vector.memset(zt[:, :], 0.0)
                ztp = zt[:, :].ap[0][0]
                nc.vector.tensor_copy(
                    Wstat[:, :].bitcast(F32R),
                    cap(zt[:, :], 0, [(ztp, 128), (0, G * 128)]))
                Pst = stage[:, :].ap[0][0]
                for q in range(4):
                    src = cap(stage[:, :], 32 * q * Pst,
                              [(Pst, 32), (32, G), (1, 32)])
                    dst = cap(Wstat[:, :], 32 * q * Pws + 32 * q,
                              [(Pws, 32), (128, G), (1, 32)])
                    nc.scalar.copy(dst.bitcast(F32R), src)

                if debug_out in ("R", "eT", "lbT", "ubT", "X", "Wstat"):
                    dbg = nc.dram_tensor(
                        "dbg", [128, R_sb[:, :].ap[0][1] and
                                (G * 1024 if debug_out == "R" else G * 32)],
                        F32, kind="ExternalOutput")
                    dump = {"R": R_sb, "eT": eT, "lbT": lbT, "ubT": ubT,
                            "X": X_sb, "Wstat": None}.get(debug_out)
                    if debug_out == "Wstat":
                        dbg2 = nc.dram_tensor("dbg2", [128, G * 128], F32,
                                              kind="ExternalOutput")
                        nc.sync.dma_start(dbg2[:, :], Wstat[:, :])
                    else:
                        nc.sync.dma_start(dbg[:, :], dump[:, :])

            tc.strict_bb_all_engine_barrier()

            # =========== ITERATIONS ===========
            with (
                tc.tile_pool(name="iscr", bufs=1) as iscr,
                tc.tile_pool(name="tr", bufs=1) as trp,
                tc.tile_pool(name="psI", bufs=2, space="PSUM") as psI,
            ):
                S_sb = iscr.tile([128, G * 32], F32, tag="S")
                Z_sb = iscr.tile([128, G * 32], F32, tag="Z")
                W_sb = iscr.tile([128, G * 32], F32, tag="W")
                Pw = W_sb[:, :].ap[0][0]
                sctA = trp.tile([128, 4096], F32, tag="sctA")
                PsA = sctA[:, :].ap[0][0]

                def one_round(last):
                    for ch in range(NCH):
                        ps = psI.tile([128, 2048], F32, tag="psit")
                        for gg in range(2):
                            g = 2 * ch + gg
                            for h in range(2):
                                o = 1024 * gg + 512 * h
                                nc.tensor.matmul(
                                    ps[:, o:o + 512],
                                    Wstat[:, 128 * g:128 * (g + 1)]
                                    .bitcast(F32R),
                                    R_sb[:, 1024 * g + 512 * h:
                                         1024 * g + 512 * (h + 1)]
                                    .bitcast(F32R),
                                    start=True, stop=True)
                        nc.vector.transpose(
                            sctA[:, 2048 * (ch % 2):2048 * (ch % 2) + 2048],
                            ps[:, :])
                        if ch % 2 == 1:
                            gat = cap(sctA[:, :], 0,
                                      [(PsA, 128), (1024, 4), (33, 32)])
                            c0 = 128 * (ch // 2)
                            nc.vector.tensor_tensor(X_sb[:, c0:c0 + 128], gat,
                                                    X_sb[:, c0:c0 + 128],
                                                    AL.add)
                        if (not last) and (ch % CPC == CPC - 1):
                            cl = ch // CPC
                            cs = slice(32 * CG * cl, 32 * CG * (cl + 1))
                            nc.vector.tensor_tensor(S_sb[:, cs], X_sb[:, cs],
                                                    U_sb[:, cs], AL.add)
                            nc.vector.tensor_max(Z_sb[:, cs], S_sb[:, cs],
                                                 lbT[:, cs])
                            nc.vector.tensor_tensor(Z_sb[:, cs], Z_sb[:, cs],
                                                    ubT[:, cs], AL.min)
                            nc.vector.tensor_sub(U_sb[:, cs], S_sb[:, cs],
                                                 Z_sb[:, cs])
                            nc.vector.scalar_tensor_tensor(
                                W_sb[:, cs], Z_sb[:, cs], 2.0, S_sb[:, cs],
                                AL.mult, AL.subtract)
                            nc.vector.scalar_tensor_tensor(
                                X_sb[:, cs], W_sb[:, cs], 1.0 / SIGMA,
                                eT[:, cs], AL.mult, AL.add)
                            # Wstat rebuild for this cluster (ACT)
                            for q in range(4):
                                src = cap(W_sb[:, :],
                                          32 * q * Pw + 32 * CG * cl,
                                          [(Pw, 32), (32, CG), (1, 32)])
                                dst = cap(Wstat[:, :],
                                          32 * q * Pws + 128 * CG * cl
                                          + 32 * q,
                                          [(Pws, 32), (128, CG), (1, 32)])
                                nc.scalar.copy(dst.bitcast(F32R), src)

                if use_for_i and n_iters > 1:
                    with tc.For_i(0, n_iters - 1, 1):
                        one_round(last=False)
                else:
                    for _ in range(n_iters - 1):
                        one_round(last=False)
                one_round(last=True)   # final: x stays in X_sb

                # =========== OUTPUT ===========
                # transpose X (T-layout) -> sample-major, then one DMA out
                xo = iscr.tile([128, G * 32], F32, tag="xo")
                nc.vector.transpose(xo[:, :], X_sb[:, :])
                Pxo = xo[:, :].ap[0][0]
                src = cap(xo[:, :], 0, [(Pxo, 128), (32, G), (1, 32)])
                dst = x_d[:, :].rearrange("(g p) w -> p g w", p=128)
                nc.sync.dma_start(dst, src)
    return nc


_NC = 8
_B = 32768
_NB = _B // _NC
_G = _NB // 128
_N_ITERS = 100
_cache = {}


def _get_nc():
    if "nc" not in _cache:
        nc = bacc.Bacc()
        build_kernel(nc, _NB, _N_ITERS, use_for_i=True)
        nc.compile()
        _cache["nc"] = nc
    return _cache["nc"]


def kernel(A, b, c, lb, ub):
    A = np.ascontiguousarray(A, np.float32)
    b = np.ascontiguousarray(b, np.float32)
    c = np.ascontiguousarray(c, np.float32)
    lb = np.ascontiguousarray(lb, np.float32)
    ub = np.ascontiguousarray(ub, np.float32)
    nc = _get_nc()
    in_maps = []
    for i in range(_NC):
        s = slice(i * _NB, (i + 1) * _NB)
        in_maps.append({"A": A[s], "b": b[s],
                        "c": c[s], "lb": lb[s], "ub": ub[s]})
    from concourse.bass_utils import run_bass_kernel_spmd
    res = run_bass_kernel_spmd(nc, in_maps, core_ids=list(range(_NC)))
    return np.concatenate([res.results[i]["x"] for i in range(_NC)], axis=0)
